# revision 10
# baseline (speedup 1.0000x reference)
"""Trainium2 Bass kernel for DTransformerLayer (strictly-causal attention +
residual + LayerNorm), distributed over 8 NeuronCores.

Sharding: core c = (batch b = c//2, row-half = c%2).
  half 0 -> query rows [0:256] u [768:1024]   (causal-balanced fold)
  half 1 -> query rows [256:768]
Each core computes all 16 heads for its 512 rows end-to-end; no cross-core
communication. Uniform SPMD program: band0 covers keys [0:512], band1 covers
keys [0:1024]; rows that need fewer keys read zeros from the host-masked
exp(state_weight) so the extra blocks contribute nothing.

Math notes:
 - scores are bounded (|q.k/8 + sw| < ~15) so softmax skips max-subtraction:
   attn = exp(qk/8)*exp(sw)*mask / sum(...). exp(sw)*mask comes from the host.
 - k shares Wq/bq (kq_same). 0.125 scale is folded into the q projection.
 - bv is folded out: ctx@Wo with v-bias == (p@v0)@Wo + rowsum*(bv@Wo); rowsum
   is 1 after normalization, so host adds bv@Wo + bo into the residual. The
   only exception is global row 0 (fully masked, rowsum 0) which the host
   recomputes exactly.
"""

import math
from contextlib import ExitStack

import numpy as np

import concourse.bacc as bacc
import concourse.bass as bass
import concourse.mybir as mybir
import concourse.tile as tile
from concourse.bass_utils import run_bass_kernel_spmd
from concourse.masks import make_identity

B, S, D, H, DK = 4, 1024, 1024, 16, 64
P = 128
NCORES = 8
ROWS = 512            # rows per core
NBANDS = 2
RB = 2                # 128-row blocks per band
NKS = (4, 8)          # key-blocks (128) per band, uniform across cores
EPS = 1e-5

f32 = mybir.dt.float32
f32r = mybir.dt.float32r

_NC_CACHE = {}


def build_nc():
    nc = bacc.Bacc("TRN2", target_bir_lowering=False, debug=False,
                   num_devices=NCORES)

    qt = nc.dram_tensor("qt", [D, ROWS], f32r, kind="ExternalInput")
    kt = nc.dram_tensor("kt", [D, S], f32r, kind="ExternalInput")
    vt = nc.dram_tensor("vt", [D, S], f32r, kind="ExternalInput")
    wq = nc.dram_tensor("wq", [D, D], f32r, kind="ExternalInput")
    wv = nc.dram_tensor("wv", [D, D], f32r, kind="ExternalInput")
    wo = nc.dram_tensor("wo", [D, D], f32r, kind="ExternalInput")
    bqv = nc.dram_tensor("bqv", [D], f32, kind="ExternalInput")
    qp = nc.dram_tensor("qp", [ROWS, D], f32, kind="ExternalInput")
    esw = nc.dram_tensor("esw", [H, ROWS, S], f32, kind="ExternalInput")
    gam = nc.dram_tensor("gam", [D], f32, kind="ExternalInput")
    bet = nc.dram_tensor("bet", [D], f32, kind="ExternalInput")
    y = nc.dram_tensor("y", [ROWS, D], f32, kind="ExternalOutput")

    with tile.TileContext(nc) as tc, ExitStack() as ctx:
        const = ctx.enter_context(tc.tile_pool(name="const", bufs=1))
        persist = ctx.enter_context(tc.tile_pool(name="persist", bufs=1))

        ident_f = const.tile([P, P], f32)
        make_identity(nc, ident_f)
        ident = const.tile([P, P], f32r)
        nc.vector.tensor_copy(ident, ident_f)
        gam_bc = const.tile([P, D], f32)
        nc.sync.dma_start(out=gam_bc, in_=gam.ap().partition_broadcast(P))
        bet_bc = const.tile([P, D], f32)
        nc.sync.dma_start(out=bet_bc, in_=bet.ap().partition_broadcast(P))
        eps_sb = const.tile([P, 1], f32)
        nc.vector.memset(eps_sb, EPS)
        bq_sb = const.tile([P, D // P], f32)
        nc.sync.dma_start(out=bq_sb, in_=bqv.ap().rearrange("(c p) -> p c", p=P))

        # persistent projected tensors
        q_sb = persist.tile([P, D // P, ROWS], f32r)    # qT [hd, rows] * 0.125
        k_sb = persist.tile([P, D // P, S], f32r)       # kT [hd, keys]
        v_sb = persist.tile([P, S // P, D], f32r)       # v  [keys, hd]
        ctxT_sb = persist.tile([P, D // P, NBANDS, 2 * P], f32r)

        nch = D // P  # 8 chunks

        # ---- phase 1a: q,k projections (needs wq, qt, kt) ----
        with tc.tile_pool(name="qk_in", bufs=1) as qk_in, \
             tc.tile_pool(name="psum_qk", bufs=2, space="PSUM") as psum_qk:
            wq_sb = qk_in.tile([P, nch, D], f32r)
            qt_sb = qk_in.tile([P, nch, ROWS], f32r)
            kt_sb = qk_in.tile([P, nch, S], f32r)
            wq_r = wq.ap().rearrange("(c p) n -> p c n", p=P)
            qt_r = qt.ap().rearrange("(c p) n -> p c n", p=P)
            kt_r = kt.ap().rearrange("(c p) n -> p c n", p=P)
            for c in range(nch):
                nc.sync.dma_start(out=wq_sb[:, c, :], in_=wq_r[:, c, :])
                nc.sync.dma_start(out=qt_sb[:, c, :], in_=qt_r[:, c, :])
                nc.sync.dma_start(out=kt_sb[:, c, :], in_=kt_r[:, c, :])

            for m in range(nch):
                ps = psum_qk.tile([P, ROWS], f32, tag="pq")
                for c in range(nch):
                    nc.tensor.matmul(ps, wq_sb[:, c, m * P:(m + 1) * P],
                                     qt_sb[:, c, :],
                                     start=(c == 0), stop=(c == nch - 1))
                # q = (psum + bq) * 0.125
                nc.vector.tensor_scalar(
                    out=q_sb[:, m, :], in0=ps,
                    scalar1=bq_sb[:, m:m + 1], scalar2=0.125,
                    op0=mybir.AluOpType.add, op1=mybir.AluOpType.mult)
            for m in range(nch):
                for g in range(2):
                    ps = psum_qk.tile([P, S // 2], f32, tag="pk")
                    for c in range(nch):
                        nc.tensor.matmul(
                            ps, wq_sb[:, c, m * P:(m + 1) * P],
                            kt_sb[:, c, g * (S // 2):(g + 1) * (S // 2)],
                            start=(c == 0), stop=(c == nch - 1))
                    nc.vector.tensor_scalar(
                        out=k_sb[:, m, g * (S // 2):(g + 1) * (S // 2)],
                        in0=ps, scalar1=bq_sb[:, m:m + 1], scalar2=None,
                        op0=mybir.AluOpType.add)

        # ---- phase 1b: v projection (needs wv, vt) ----
        with tc.tile_pool(name="v_in", bufs=1) as v_in, \
             tc.tile_pool(name="psum_v", bufs=2, space="PSUM") as psum_v:
            wv_sb = v_in.tile([P, nch, D], f32r)
            vt_sb = v_in.tile([P, nch, S], f32r)
            wv_r = wv.ap().rearrange("(c p) n -> p c n", p=P)
            vt_r = vt.ap().rearrange("(c p) n -> p c n", p=P)
            for c in range(nch):
                nc.sync.dma_start(out=wv_sb[:, c, :], in_=wv_r[:, c, :])
                nc.sync.dma_start(out=vt_sb[:, c, :], in_=vt_r[:, c, :])
            for sc in range(S // P):
                for g in range(2):
                    ps = psum_v.tile([P, D // 2], f32, tag="pv")
                    for c in range(nch):
                        nc.tensor.matmul(
                            ps, vt_sb[:, c, sc * P:(sc + 1) * P],
                            wv_sb[:, c, g * (D // 2):(g + 1) * (D // 2)],
                            start=(c == 0), stop=(c == nch - 1))
                    nc.scalar.copy(
                        v_sb[:, sc, g * (D // 2):(g + 1) * (D // 2)], ps)

        # ---- phase 2: attention + output projection + layernorm ----
        with tc.tile_pool(name="attn", bufs=1) as attn_pool, \
             tc.tile_pool(name="stream", bufs=2) as stream, \
             tc.tile_pool(name="qpstream", bufs=1) as qpstream, \
             tc.tile_pool(name="pstream", bufs=4) as pstream, \
             tc.tile_pool(name="scratch", bufs=1) as scratch, \
             tc.tile_pool(name="small", bufs=6) as small, \
             tc.tile_pool(name="ps_sc", bufs=3, space="PSUM") as ps_sc, \
             tc.tile_pool(name="ps_at", bufs=2, space="PSUM") as ps_at, \
             tc.tile_pool(name="ps_cx", bufs=2, space="PSUM") as ps_cx, \
             tc.tile_pool(name="ps_op", bufs=1, space="PSUM") as ps_op:

            wo_sb = attn_pool.tile([P, nch, D], f32r)
            wo_r = wo.ap().rearrange("(c p) n -> p c n", p=P)
            for c in range(nch):
                nc.sync.dma_start(out=wo_sb[:, c, :], in_=wo_r[:, c, :])

            esw_ap = esw.ap()
            qp_r = qp.ap().rearrange("(c p) n -> p c n", p=P)
            y_r = y.ap().rearrange("(c p) n -> p c n", p=P)

            def softmax_stage(band, h):
                """scores -> exp -> *esw (+row sums) -> normalize.
                Returns the normalized p tiles (one per rb)."""
                nk = NKS[band]
                W = nk * P
                p_rb = []
                sums = small.tile([P, RB], f32, tag="sums")
                for rb in range(RB):
                    r0 = band * 256 + rb * P
                    hp = (h % 2) * DK
                    hc = h // 2
                    p_t = pstream.tile([P, NKS[1] * P], f32r, tag="p")
                    esw_t = stream.tile([P, NKS[1] * P], f32, tag="esw")
                    nc.sync.dma_start(out=esw_t[:, :W],
                                      in_=esw_ap[h, r0:r0 + P, 0:W])
                    for g in range(W // 512):
                        sl = slice(g * 512, (g + 1) * 512)
                        sc_ps = ps_sc.tile([P, 512], f32, tag="sc")
                        nc.tensor.matmul(
                            sc_ps,
                            q_sb[hp:hp + DK, hc, r0:r0 + P],
                            k_sb[hp:hp + DK, hc, sl],
                            start=True, stop=True)
                        nc.scalar.activation(
                            out=p_t[:, sl], in_=sc_ps,
                            func=mybir.ActivationFunctionType.Exp)
                    nc.vector.scalar_tensor_tensor(
                        out=p_t[:, :W], in0=p_t[:, :W], scalar=1.0,
                        in1=esw_t[:, :W],
                        op0=mybir.AluOpType.mult, op1=mybir.AluOpType.mult,
                        accum_out=sums[:, rb:rb + 1])
                    p_rb.append(p_t)
                nc.vector.tensor_scalar_add(sums, sums, 1e-30)
                nc.vector.reciprocal(sums, sums)
                for rb in range(RB):
                    W = NKS[band] * P
                    nc.vector.tensor_scalar(
                        out=p_rb[rb][:, :W],
                        in0=p_rb[rb][:, :W],
                        scalar1=sums[:, rb:rb + 1], scalar2=None,
                        op0=mybir.AluOpType.mult)
                return p_rb

            def ctx_stage(band, h, p_rb):
                """transpose p, accumulate ctxT over key blocks, store."""
                nk = NKS[band]
                hp = (h % 2) * DK
                hc = h // 2
                cx_ps = ps_cx.tile([DK, 2 * P], f32, tag="cx")
                for kb in range(nk):
                    at_ps = ps_at.tile([P, 2 * P], f32r, tag="at")
                    for rb in range(RB):
                        nc.tensor.transpose(
                            at_ps[:, rb * P:(rb + 1) * P],
                            p_rb[rb][:, kb * P:(kb + 1) * P],
                            ident)
                    at_sb = pstream.tile([P, 2 * P], f32r, tag="at_sb")
                    if kb % 2 == 0:
                        nc.scalar.copy(at_sb, at_ps)
                    else:
                        nc.vector.tensor_copy(at_sb, at_ps)
                    nc.tensor.matmul(
                        cx_ps,
                        v_sb[:, kb, h * DK:(h + 1) * DK],
                        at_sb, start=(kb == 0), stop=(kb == nk - 1))
                if hp == 0:
                    nc.vector.tensor_copy(ctxT_sb[0:DK, hc, band, :], cx_ps)
                else:
                    # engines cannot shift partitions; stage in SBUF and let
                    # a DMA remap partitions 0:64 -> 64:128
                    stage_sb = pstream.tile([DK, 2 * P], f32r, tag="cstg")
                    nc.vector.tensor_copy(stage_sb, cx_ps)
                    nc.sync.dma_start(out=ctxT_sb[DK:P, hc, band, :],
                                      in_=stage_sb)

            def out_stage(band, rb):
                """output projection + residual + layernorm for a row chunk."""
                rc = band * RB + rb
                x_t = stream.tile([P, D], f32, tag="x")
                qp_t = qpstream.tile([P, D], f32, tag="qp")
                nc.sync.dma_start(out=qp_t, in_=qp_r[:, rc, :])
                xs = small.tile([P, 2], f32, tag="xs")
                for g in range(2):
                    op_ps = ps_op.tile([P, D // 2], f32, tag="op")
                    for c in range(nch):
                        nc.tensor.matmul(
                            op_ps,
                            ctxT_sb[:, c, band, rb * P:(rb + 1) * P],
                            wo_sb[:, c, g * (D // 2):(g + 1) * (D // 2)],
                            start=(c == 0), stop=(c == nch - 1))
                    nc.vector.scalar_tensor_tensor(
                        out=x_t[:, g * (D // 2):(g + 1) * (D // 2)],
                        in0=op_ps, scalar=1.0,
                        in1=qp_t[:, g * (D // 2):(g + 1) * (D // 2)],
                        op0=mybir.AluOpType.mult, op1=mybir.AluOpType.add,
                        accum_out=xs[:, g:g + 1])
                st = small.tile([P, 4], f32, tag="st")
                nc.vector.tensor_tensor(out=st[:, 0:1], in0=xs[:, 0:1],
                                        in1=xs[:, 1:2],
                                        op=mybir.AluOpType.add)
                nc.vector.tensor_scalar_mul(st[:, 0:1], st[:, 0:1],
                                            1.0 / D)  # mu
                sq_t = scratch.tile([P, D], f32, tag="sq")
                nc.scalar.activation(out=sq_t, in_=x_t,
                                     func=mybir.ActivationFunctionType.Square,
                                     accum_out=st[:, 1:2])  # sum(x^2)
                nc.vector.tensor_scalar_mul(st[:, 1:2], st[:, 1:2], 1.0 / D)
                nc.vector.tensor_tensor(out=st[:, 2:3], in0=st[:, 0:1],
                                        in1=st[:, 0:1],
                                        op=mybir.AluOpType.mult)  # mu^2
                nc.vector.tensor_tensor(out=st[:, 1:2], in0=st[:, 1:2],
                                        in1=st[:, 2:3],
                                        op=mybir.AluOpType.subtract)  # var
                nc.scalar.activation(out=st[:, 1:2], in_=st[:, 1:2],
                                     func=mybir.ActivationFunctionType.Sqrt,
                                     bias=eps_sb[:, 0:1])
                nc.vector.reciprocal(st[:, 1:2], st[:, 1:2])  # rstd
                nc.vector.tensor_scalar(
                    out=x_t, in0=x_t,
                    scalar1=st[:, 0:1], scalar2=st[:, 1:2],
                    op0=mybir.AluOpType.subtract, op1=mybir.AluOpType.mult)
                nc.vector.scalar_tensor_tensor(
                    out=x_t, in0=x_t, scalar=1.0, in1=gam_bc,
                    op0=mybir.AluOpType.mult, op1=mybir.AluOpType.mult)
                y_t = stream.tile([P, D], f32, tag="y")
                nc.vector.tensor_tensor(out=y_t, in0=x_t, in1=bet_bc,
                                        op=mybir.AluOpType.add)
                nc.sync.dma_start(out=y_r[:, rc, :], in_=y_t)

            # software-pipelined head loop: softmax of head h+1 is emitted
            # before the transpose/ctx stage of head h so every engine has
            # independent work while the per-head chain drains.
            for band in range(NBANDS):
                p_cur = softmax_stage(band, 0)
                for h in range(H):
                    p_next = softmax_stage(band, h + 1) if h + 1 < H else None
                    ctx_stage(band, h, p_cur)
                    p_cur = p_next
                for rb in range(RB):
                    out_stage(band, rb)

    nc.compile()
    return nc


def _get_nc():
    if "nc" not in _NC_CACHE:
        _NC_CACHE["nc"] = build_nc()
    return _NC_CACHE["nc"]


def _row_index(half):
    if half == 0:
        return np.r_[0:256, 768:1024]
    return np.r_[256:768]


def prepare_in_maps(query, key, values, state_weight, Wq, bq, Wv, bv, Wo, bo,
                    gamma, beta):
    f = np.float32
    query = np.asarray(query, f)
    key = np.asarray(key, f)
    values = np.asarray(values, f)
    sw = np.asarray(state_weight, f)[0]              # [H, S, S]
    Wq = np.ascontiguousarray(np.asarray(Wq, f))
    Wv = np.ascontiguousarray(np.asarray(Wv, f))
    Wo = np.ascontiguousarray(np.asarray(Wo, f))
    bq = np.asarray(bq, f)
    bo_eff = (np.asarray(bo, f) + np.asarray(bv, f) @ Wo).astype(f)
    gamma = np.asarray(gamma, f)
    beta = np.asarray(beta, f)

    mask = np.tril(np.ones((S, S), f), -1)
    esw_full = (np.exp(sw) * mask[None]).astype(f)   # [H, S, S]

    in_maps = []
    for c in range(NCORES):
        b, half = divmod(c, 2)
        ridx = _row_index(half)
        qT = np.ascontiguousarray(query[b].T)
        in_maps.append({
            "qt": np.ascontiguousarray(qT[:, ridx]),
            "kt": np.ascontiguousarray(key[b].T),
            "vt": np.ascontiguousarray(values[b].T),
            "wq": Wq, "wv": Wv, "wo": Wo,
            "bqv": bq,
            "qp": np.ascontiguousarray(query[b][ridx] + bo_eff[None, :]),
            "esw": np.ascontiguousarray(esw_full[:, ridx, :]),
            "gam": gamma, "bet": beta,
        })
    return in_maps


def assemble_output(results, query, bo, gamma, beta):
    f = np.float32
    out = np.empty((B, S, D), f)
    for c in range(NCORES):
        b, half = divmod(c, 2)
        out[b, _row_index(half)] = results[c]["y"]
    # global row 0 is fully masked: ctx = 0 exactly (no bv@Wo term)
    q0 = np.asarray(query, f)[:, 0, :] + np.asarray(bo, f)[None, :]
    mu = q0.mean(axis=1, keepdims=True)
    var = ((q0 - mu) ** 2).mean(axis=1, keepdims=True)
    out[:, 0, :] = ((q0 - mu) / np.sqrt(var + EPS) * np.asarray(gamma, f)
                    + np.asarray(beta, f))
    return out


def kernel(query, key, values, lens, state_weight, Wq, bq, Wv, bv, Wo, bo,
           gamma, beta, _trace=False):
    in_maps = prepare_in_maps(query, key, values, state_weight, Wq, bq, Wv,
                              bv, Wo, bo, gamma, beta)
    nc = _get_nc()
    res = run_bass_kernel_spmd(nc, in_maps, core_ids=list(range(NCORES)),
                               trace=_trace)
    out = assemble_output(res.results, query, bo, gamma, beta)
    if _trace:
        return out, res
    return out
